# revision 3
# baseline (speedup 1.0000x reference)
"""V-A: 4-queue SWDGE dma_gather + min-only stage1.

Shard over G (each of 8 cores handles 256 g'; x replicated). Partition p
holds g' in {2p, 2p+1}; grp c = gl*32 + s (gl = g' parity, s). 64 gather
calls: call (lb, l) gathers literal l of grps 8lb..8lb+7 for all 128
partitions (1024 idx, list position j = i*128 + p, i = grp offset).
Calls round-robin over SWDGE queues 0-3 so descriptor generation runs on
all four Q7 core pairs concurrently. softand over l is computed as an
elementwise min of the 8 per-l tiles (contiguous DVE ops); softor over s
stays an exact logsumexp per gl half.
"""

import numpy as np

import concourse.bacc as bacc
import concourse.bass as bass
import concourse.tile as tile
from concourse import mybir
from concourse.bass_utils import run_bass_kernel_spmd

B, G, S, L = 64, 2048, 32, 8
NCORES = 8
GSH = G // NCORES  # 256 g' per core
NIDX = 1024  # indices per dma_gather call
NQ = 4  # SWDGE queues
GRP_PER_PART = GSH // 128 * S  # 64 grps (gl, s) per partition
NBATCH = GRP_PER_PART // 8  # 8 batches of 8 grps

_nc_cache = None
last_result = None


def _v(t, dims, off=0):
    return bass.AP(tensor=t.tensor, offset=t.offset + off, ap=[list(t.ap[0])] + dims)


def _stage2(nc, small, vv, c_out, gl):
    """softor over s for half gl of vv; writes c_out columns [gl*64,(gl+1)*64)."""
    f32 = mybir.dt.float32
    off = gl * 32 * B
    vm = small.tile([128, B], f32, tag="vm")
    nc.vector.tensor_reduce(
        out=vm,
        in_=_v(vv, [[1, B], [B, 32]], off),  # [b, s]
        axis=mybir.AxisListType.X,
        op=mybir.AluOpType.max,
    )
    d2 = small.tile([128, 32, B], f32, tag="d2")
    nc.vector.tensor_tensor(
        out=d2,
        in0=_v(vv, [[B, 32], [1, B]], off),  # [s, b]
        in1=_v(vm, [[0, 32], [1, B]]),  # M bcast over s
        op=mybir.AluOpType.subtract,
    )  # v - M (<= 0)
    e2 = small.tile([128, 32, B], f32, tag="e2")
    nc.scalar.activation(
        out=e2, in_=d2, func=mybir.ActivationFunctionType.Exp, scale=1000.0
    )
    s2 = small.tile([128, B], f32, tag="s2")
    nc.vector.tensor_reduce(
        out=s2,
        in_=_v(e2, [[1, B], [B, 32]]),  # [b, s]
        axis=mybir.AxisListType.X,
        op=mybir.AluOpType.add,
    )
    l2 = small.tile([128, B], f32, tag="l2")
    nc.scalar.activation(out=l2, in_=s2, func=mybir.ActivationFunctionType.Ln)
    l2s = small.tile([128, B], f32, tag="l2s")
    nc.scalar.activation(
        out=l2s, in_=l2, func=mybir.ActivationFunctionType.Copy, scale=0.001
    )
    cf = small.tile([128, B], f32, tag="cf")
    nc.vector.tensor_tensor(out=cf, in0=vm, in1=l2s, op=mybir.AluOpType.add)
    nc.sync.dma_start(out=c_out[:, gl * B : (gl + 1) * B], in_=cf)


def _build_nc():
    f32 = mybir.dt.float32
    nc = bacc.Bacc("TRN2", target_bir_lowering=False, num_swdge_queues=NQ)
    tbl_in = nc.dram_tensor("tbl", [G, B], f32, kind="ExternalInput")  # x.T
    idx_in = nc.dram_tensor(
        "idx", [128, L * NBATCH * NIDX // 16], mybir.dt.int16, kind="ExternalInput"
    )
    c_out = nc.dram_tensor("c", [128, 128], f32, kind="ExternalOutput")

    with tile.TileContext(nc) as tc:
        with (
            tc.tile_pool(name="singles", bufs=1) as singles,
            tc.tile_pool(name="gath", bufs=2) as gath,
            tc.tile_pool(name="work", bufs=2) as work,
            tc.tile_pool(name="small", bufs=2) as small,
        ):
            idxs = singles.tile([128, L * NBATCH * NIDX // 16], mybir.dt.int16)
            first_cols = L * (NIDX // 16)
            nc.sync.dma_start(out=idxs[:, :first_cols], in_=idx_in[:, :first_cols])
            nc.sync.dma_start(out=idxs[:, first_cols:], in_=idx_in[:, first_cols:])
            vv = singles.tile([128, GRP_PER_PART, B], f32)  # min over l
            for lb in range(NBATCH):
                gt = gath.tile([128, L, 8, B], f32, tag="gt")
                for l in range(L):
                    c = lb * L + l
                    nc.gpsimd.dma_gather(
                        gt[:, l, :, :],
                        tbl_in[:, :],
                        idxs[:, c * (NIDX // 16) : (c + 1) * (NIDX // 16)],
                        num_idxs=NIDX,
                        num_idxs_reg=NIDX,
                        elem_size=B,
                        queue_num=c % NQ,
                    )
                # elementwise min over the 8 per-l tiles (contiguous)
                m0 = work.tile([128, 8, B], f32, tag="m0")
                nc.vector.tensor_tensor(
                    out=m0,
                    in0=gt[:, 0, :, :],
                    in1=gt[:, 1, :, :],
                    op=mybir.AluOpType.min,
                )
                m1 = work.tile([128, 8, B], f32, tag="m1")
                nc.vector.tensor_tensor(
                    out=m1,
                    in0=gt[:, 2, :, :],
                    in1=gt[:, 3, :, :],
                    op=mybir.AluOpType.min,
                )
                m2 = work.tile([128, 8, B], f32, tag="m2")
                nc.vector.tensor_tensor(
                    out=m2,
                    in0=gt[:, 4, :, :],
                    in1=gt[:, 5, :, :],
                    op=mybir.AluOpType.min,
                )
                m3 = work.tile([128, 8, B], f32, tag="m3")
                nc.vector.tensor_tensor(
                    out=m3,
                    in0=gt[:, 6, :, :],
                    in1=gt[:, 7, :, :],
                    op=mybir.AluOpType.min,
                )
                m4 = work.tile([128, 8, B], f32, tag="m4")
                nc.vector.tensor_tensor(out=m4, in0=m0, in1=m1, op=mybir.AluOpType.min)
                m5 = work.tile([128, 8, B], f32, tag="m5")
                nc.vector.tensor_tensor(out=m5, in0=m2, in1=m3, op=mybir.AluOpType.min)
                nc.vector.tensor_tensor(
                    out=vv[:, lb * 8 : (lb + 1) * 8, :],
                    in0=m4,
                    in1=m5,
                    op=mybir.AluOpType.min,
                )
                if (lb + 1) * 8 % 32 == 0:
                    _stage2(nc, small, vv, c_out, (lb + 1) * 8 // 32 - 1)
    nc.finalize()
    return nc


def _prep_inputs(x: np.ndarray, I_i: np.ndarray):
    """Host-side layout: x transposed; per-core wrapped idx tensors."""
    tbl = np.ascontiguousarray(x.astype(np.float32, copy=False).T)  # [G, B]
    idx_maps = []
    I = np.asarray(I_i)
    for k in range(NCORES):
        Ik = I[k * GSH : (k + 1) * GSH]  # [256, 32, 8] values in [0, G)
        # partition p holds g' = 2p + gl; grp c = gl*32 + s
        Ikr = Ik.reshape(128, 2, S, L)  # [p, gl, s, l]
        # call (lb, l): list position j = i*128 + p gathers grp c=8lb+i of p
        grp = np.transpose(Ikr, (1, 2, 3, 0)).reshape(GRP_PER_PART, L, 128)  # [c,l,p]
        cl = grp.reshape(NBATCH, 8, L, 128)  # [lb, i, l, p]
        flat = np.transpose(cl, (0, 2, 1, 3)).reshape(NBATCH * L, NIDX)  # [call, j]
        w = flat.reshape(NBATCH * L, NIDX // 16, 16)  # [call, t, q%16]
        w = np.transpose(w, (2, 0, 1)).reshape(16, NBATCH * L * (NIDX // 16))
        idx = np.tile(w, (8, 1)).astype(np.int16)  # replicate to 128 partitions
        idx_maps.append(idx)
    return tbl, idx_maps


def kernel(x: np.ndarray, I_i: np.ndarray) -> np.ndarray:
    global _nc_cache, last_result
    if _nc_cache is None:
        _nc_cache = _build_nc()
    nc = _nc_cache
    tbl, idx_maps = _prep_inputs(x, I_i)
    in_maps = [{"tbl": tbl, "idx": idx_maps[k]} for k in range(NCORES)]
    res = run_bass_kernel_spmd(nc, in_maps, core_ids=list(range(NCORES)))
    last_result = res
    C = np.empty((B, G), dtype=np.float32)
    for k in range(NCORES):
        o = res.results[k]["c"].reshape(128, 2, B)  # [p, gl, b]
        C[:, k * GSH : (k + 1) * GSH] = np.transpose(o, (2, 0, 1)).reshape(B, GSH)
    return C


# revision 4
# speedup vs baseline: 1.0658x; 1.0658x over previous
"""Final: 4-queue SWDGE dma_gather (NIDX=1024) + pure min/max compute.

Shard over G (each of 8 cores handles 256 g'; x.T replicated). Partition p
holds g' in {2p, 2p+1}; grp c = gl*32 + s (gl = g' parity, s). 32 gather
calls: call (lb, l) gathers literal l of grps 16lb..16lb+16 for all 128
partitions (2048 idx, list position j = i*128 + p). Calls round-robin over
SWDGE queues 0-3 (queue = c % 4); each queue's idx data lives only in its
column range [c*128, (c+1)*128) of a [128, 4096] int16 idx tensor
(16-wrap replicated to all partitions), DMA'd in 4 batch-aligned splits.

softand over l = elementwise fp32 min tree (contiguous DVE ops); softor
over s = pure max tree (error bounded by gamma*ln32 = 3.5e-3 and partially
cancelling the min-side bias; no activation functions at all).
"""

import numpy as np

import concourse.bacc as bacc
import concourse.bass as bass
import concourse.tile as tile
from concourse import mybir
from concourse.bass_utils import run_bass_kernel_spmd

B, G, S, L = 64, 2048, 32, 8
NCORES = 8
GSH = G // NCORES  # 256 g' per core
NIDX = 1024  # indices per dma_gather call (ucode scratch-safe)
NQ = 4  # SWDGE queues
NBATCH = 8  # batches of 8 grps; NBATCH*L = 64 calls
GRP_PER_BATCH = 8
COLS_PER_CALL = NIDX // 16  # 128 idx columns
IDX_COLS = NBATCH * L * COLS_PER_CALL  # 4096

_nc_cache = None
last_result = None


def _v(t, dims, off=0):
    return bass.AP(tensor=t.tensor, offset=t.offset + off, ap=[list(t.ap[0])] + dims)


def _maxtree(nc, work, vv, c_out, gl):
    """pure softor~max over s for half gl; writes c_out cols [gl*64,(gl+1)*64)."""
    f32 = mybir.dt.float32
    off = gl * 32 * B
    t16 = work.tile([128, 16, B], f32, tag="t16")
    nc.vector.tensor_tensor(
        out=t16,
        in0=_v(vv, [[B, 16], [1, B]], off),
        in1=_v(vv, [[B, 16], [1, B]], off + 16 * B),
        op=mybir.AluOpType.max,
    )
    t8 = work.tile([128, 8, B], f32, tag="t8")
    nc.vector.tensor_tensor(
        out=t8, in0=t16[:, 0:8], in1=t16[:, 8:16], op=mybir.AluOpType.max
    )
    t4 = work.tile([128, 4, B], f32, tag="t4")
    nc.vector.tensor_tensor(
        out=t4, in0=t8[:, 0:4], in1=t8[:, 4:8], op=mybir.AluOpType.max
    )
    t2 = work.tile([128, 2, B], f32, tag="t2")
    nc.vector.tensor_tensor(
        out=t2, in0=t4[:, 0:2], in1=t4[:, 2:4], op=mybir.AluOpType.max
    )
    t1 = work.tile([128, B], f32, tag="t1")
    nc.vector.tensor_tensor(
        out=t1, in0=t2[:, 0], in1=t2[:, 1], op=mybir.AluOpType.max
    )
    nc.sync.dma_start(out=c_out[:, gl * B : (gl + 1) * B], in_=t1)


def _build_nc():
    f32 = mybir.dt.float32
    nc = bacc.Bacc("TRN2", target_bir_lowering=False, num_swdge_queues=NQ)
    tbl_in = nc.dram_tensor("tbl", [G, B], f32, kind="ExternalInput")  # x.T
    idx_in = nc.dram_tensor("idx", [128, IDX_COLS], mybir.dt.int16, kind="ExternalInput")
    c_out = nc.dram_tensor("c", [128, 128], f32, kind="ExternalOutput")

    with tile.TileContext(nc) as tc:
        with (
            tc.tile_pool(name="singles", bufs=1) as singles,
            tc.tile_pool(name="gath", bufs=2) as gath,
            tc.tile_pool(name="work", bufs=2) as work,
        ):
            idxs = singles.tile([128, IDX_COLS], mybir.dt.int16)
            # 4 column-splits: split s covers queue-call index i = 2s, 2s+1
            for sp in range(4):
                c0 = sp * (IDX_COLS // 4)
                c1 = (sp + 1) * (IDX_COLS // 4)
                nc.sync.dma_start(out=idxs[:, c0:c1], in_=idx_in[:, c0:c1])
            vv = singles.tile([128, 2 * S, B], f32)  # min over l per grp
            for lb in range(NBATCH):
                gt = gath.tile([128, L, GRP_PER_BATCH, B], f32, tag="gt")
                for l in range(L):
                    c = lb * L + l
                    nc.gpsimd.dma_gather(
                        gt[:, l, :, :],
                        tbl_in[:, :],
                        idxs[:, c * COLS_PER_CALL : (c + 1) * COLS_PER_CALL],
                        num_idxs=NIDX,
                        num_idxs_reg=NIDX,
                        elem_size=B,
                        queue_num=c % NQ,
                    )
                # elementwise min tree over the 8 per-l tiles (contiguous)
                m0 = work.tile([128, GRP_PER_BATCH, B], f32, tag="m0")
                nc.vector.tensor_tensor(
                    out=m0, in0=gt[:, 0], in1=gt[:, 1], op=mybir.AluOpType.min
                )
                m1 = work.tile([128, GRP_PER_BATCH, B], f32, tag="m1")
                nc.vector.tensor_tensor(
                    out=m1, in0=gt[:, 2], in1=gt[:, 3], op=mybir.AluOpType.min
                )
                m2 = work.tile([128, GRP_PER_BATCH, B], f32, tag="m2")
                nc.vector.tensor_tensor(
                    out=m2, in0=gt[:, 4], in1=gt[:, 5], op=mybir.AluOpType.min
                )
                m3 = work.tile([128, GRP_PER_BATCH, B], f32, tag="m3")
                nc.vector.tensor_tensor(
                    out=m3, in0=gt[:, 6], in1=gt[:, 7], op=mybir.AluOpType.min
                )
                m4 = work.tile([128, GRP_PER_BATCH, B], f32, tag="m4")
                nc.vector.tensor_tensor(out=m4, in0=m0, in1=m1, op=mybir.AluOpType.min)
                m5 = work.tile([128, GRP_PER_BATCH, B], f32, tag="m5")
                nc.vector.tensor_tensor(out=m5, in0=m2, in1=m3, op=mybir.AluOpType.min)
                nc.vector.tensor_tensor(
                    out=vv[:, lb * GRP_PER_BATCH : (lb + 1) * GRP_PER_BATCH, :],
                    in0=m4,
                    in1=m5,
                    op=mybir.AluOpType.min,
                )
                if lb % 4 == 3:
                    _maxtree(nc, work, vv, c_out, lb // 4)
    nc.finalize()
    return nc


def _prep_inputs(x: np.ndarray, I_i: np.ndarray):
    """Host-side layout: x transposed; per-core per-queue-window idx tensors."""
    tbl = np.ascontiguousarray(x.astype(np.float32, copy=False).T)  # [G, B]
    idx_maps = []
    I = np.asarray(I_i)
    for k in range(NCORES):
        Ik = I[k * GSH : (k + 1) * GSH]  # [256, 32, 8] values in [0, G)
        Ikr = Ik.reshape(128, 2, S, L)  # [p, gl, s, l]
        # grp c2 = gl*32 + s; call (lb, l) covers grps 16lb..16lb+16
        grp = np.transpose(Ikr, (1, 2, 3, 0)).reshape(2 * S, L, 128)  # [c2, l, p]
        idx_w = np.empty((16, IDX_COLS), dtype=np.int16)
        for c in range(NBATCH * L):
            lb, l = c // L, c % L
            flat = grp[
                lb * GRP_PER_BATCH : (lb + 1) * GRP_PER_BATCH, l, :
            ].reshape(NIDX)  # j = i2*128+p
            W = flat.reshape(COLS_PER_CALL, 16).T.astype(np.int16)  # [r, col]
            idx_w[:, c * COLS_PER_CALL : (c + 1) * COLS_PER_CALL] = W
        idx_maps.append(np.tile(idx_w, (8, 1)))
    return tbl, idx_maps


def kernel(x: np.ndarray, I_i: np.ndarray) -> np.ndarray:
    global _nc_cache, last_result
    if _nc_cache is None:
        _nc_cache = _build_nc()
    nc = _nc_cache
    tbl, idx_maps = _prep_inputs(x, I_i)
    in_maps = [{"tbl": tbl, "idx": idx_maps[k]} for k in range(NCORES)]
    res = run_bass_kernel_spmd(nc, in_maps, core_ids=list(range(NCORES)))
    last_result = res
    C = np.empty((B, G), dtype=np.float32)
    for k in range(NCORES):
        o = res.results[k]["c"].reshape(128, 2, B)  # [p, gl, b]
        C[:, k * GSH : (k + 1) * GSH] = np.transpose(o, (2, 0, 1)).reshape(B, GSH)
    return C


# revision 5
# speedup vs baseline: 1.0681x; 1.0021x over previous
"""Final: 4-queue SWDGE dma_gather (NIDX=1024) + pure min/max compute.

Shard over G (each of 8 cores handles 256 g'; x.T replicated). Partition p
holds g' in {2p, 2p+1}; grp c = gl*32 + s (gl = g' parity, s). 32 gather
calls: call (lb, l) gathers literal l of grps 16lb..16lb+16 for all 128
partitions (2048 idx, list position j = i*128 + p). Calls round-robin over
SWDGE queues 0-3 (queue = c % 4); each queue's idx data lives only in its
column range [c*128, (c+1)*128) of a [128, 4096] int16 idx tensor
(16-wrap replicated to all partitions), DMA'd in 4 batch-aligned splits.

softand over l = elementwise fp32 min tree (contiguous DVE ops); softor
over s = pure max tree (error bounded by gamma*ln32 = 3.5e-3 and partially
cancelling the min-side bias; no activation functions at all).
"""

import numpy as np

import concourse.bacc as bacc
import concourse.bass as bass
import concourse.tile as tile
from concourse import mybir
from concourse.bass_utils import run_bass_kernel_spmd

B, G, S, L = 64, 2048, 32, 8
NCORES = 8
GSH = G // NCORES  # 256 g' per core
NIDX = 1024  # indices per dma_gather call (ucode scratch-safe)
NQ = 4  # SWDGE queues
NBATCH = 8  # batches of 8 grps; NBATCH*L = 64 calls
GRP_PER_BATCH = 8
COLS_PER_CALL = NIDX // 16  # 128 idx columns
IDX_COLS = NBATCH * L * COLS_PER_CALL  # 4096

_nc_cache = None
last_result = None


def _v(t, dims, off=0):
    return bass.AP(tensor=t.tensor, offset=t.offset + off, ap=[list(t.ap[0])] + dims)


def _maxtree(nc, work, vv, c_out, gl):
    """pure softor~max over s for half gl; writes c_out cols [gl*64,(gl+1)*64)."""
    f32 = mybir.dt.float32
    off = gl * 32 * B
    t16 = work.tile([128, 16, B], f32, tag="t16")
    nc.vector.tensor_tensor(
        out=t16,
        in0=_v(vv, [[B, 16], [1, B]], off),
        in1=_v(vv, [[B, 16], [1, B]], off + 16 * B),
        op=mybir.AluOpType.max,
    )
    t8 = work.tile([128, 8, B], f32, tag="t8")
    nc.vector.tensor_tensor(
        out=t8, in0=t16[:, 0:8], in1=t16[:, 8:16], op=mybir.AluOpType.max
    )
    t4 = work.tile([128, 4, B], f32, tag="t4")
    nc.vector.tensor_tensor(
        out=t4, in0=t8[:, 0:4], in1=t8[:, 4:8], op=mybir.AluOpType.max
    )
    t2 = work.tile([128, 2, B], f32, tag="t2")
    nc.vector.tensor_tensor(
        out=t2, in0=t4[:, 0:2], in1=t4[:, 2:4], op=mybir.AluOpType.max
    )
    t1 = work.tile([128, B], f32, tag="t1")
    nc.vector.tensor_tensor(
        out=t1, in0=t2[:, 0], in1=t2[:, 1], op=mybir.AluOpType.max
    )
    nc.sync.dma_start(out=c_out[:, gl * B : (gl + 1) * B], in_=t1)


def _build_nc():
    f32 = mybir.dt.float32
    nc = bacc.Bacc("TRN2", target_bir_lowering=False, num_swdge_queues=NQ)
    tbl_in = nc.dram_tensor("tbl", [G, B], f32, kind="ExternalInput")  # x.T
    idx_in = nc.dram_tensor("idx", [128, IDX_COLS], mybir.dt.int16, kind="ExternalInput")
    c_out = nc.dram_tensor("c", [128, 128], f32, kind="ExternalOutput")

    with tile.TileContext(nc) as tc:
        with (
            tc.tile_pool(name="singles", bufs=1) as singles,
            tc.tile_pool(name="gath", bufs=4) as gath,
            tc.tile_pool(name="work", bufs=3) as work,
        ):
            idxs = singles.tile([128, IDX_COLS], mybir.dt.int16)
            # 4 column-splits: split s covers queue-call index i = 2s, 2s+1
            for sp in range(4):
                c0 = sp * (IDX_COLS // 4)
                c1 = (sp + 1) * (IDX_COLS // 4)
                nc.sync.dma_start(out=idxs[:, c0:c1], in_=idx_in[:, c0:c1])
            vv = singles.tile([128, 2 * S, B], f32)  # min over l per grp
            for lb in range(NBATCH):
                gt = gath.tile([128, L, GRP_PER_BATCH, B], f32, tag="gt")
                for l in range(L):
                    c = lb * L + l
                    nc.gpsimd.dma_gather(
                        gt[:, l, :, :],
                        tbl_in[:, :],
                        idxs[:, c * COLS_PER_CALL : (c + 1) * COLS_PER_CALL],
                        num_idxs=NIDX,
                        num_idxs_reg=NIDX,
                        elem_size=B,
                        queue_num=c % NQ,
                    )
                # elementwise min tree over the 8 per-l tiles (contiguous)
                m0 = work.tile([128, GRP_PER_BATCH, B], f32, tag="m0")
                nc.vector.tensor_tensor(
                    out=m0, in0=gt[:, 0], in1=gt[:, 1], op=mybir.AluOpType.min
                )
                m1 = work.tile([128, GRP_PER_BATCH, B], f32, tag="m1")
                nc.vector.tensor_tensor(
                    out=m1, in0=gt[:, 2], in1=gt[:, 3], op=mybir.AluOpType.min
                )
                m2 = work.tile([128, GRP_PER_BATCH, B], f32, tag="m2")
                nc.vector.tensor_tensor(
                    out=m2, in0=gt[:, 4], in1=gt[:, 5], op=mybir.AluOpType.min
                )
                m3 = work.tile([128, GRP_PER_BATCH, B], f32, tag="m3")
                nc.vector.tensor_tensor(
                    out=m3, in0=gt[:, 6], in1=gt[:, 7], op=mybir.AluOpType.min
                )
                m4 = work.tile([128, GRP_PER_BATCH, B], f32, tag="m4")
                nc.vector.tensor_tensor(out=m4, in0=m0, in1=m1, op=mybir.AluOpType.min)
                m5 = work.tile([128, GRP_PER_BATCH, B], f32, tag="m5")
                nc.vector.tensor_tensor(out=m5, in0=m2, in1=m3, op=mybir.AluOpType.min)
                nc.vector.tensor_tensor(
                    out=vv[:, lb * GRP_PER_BATCH : (lb + 1) * GRP_PER_BATCH, :],
                    in0=m4,
                    in1=m5,
                    op=mybir.AluOpType.min,
                )
                if lb % 4 == 3:
                    _maxtree(nc, work, vv, c_out, lb // 4)
    nc.finalize()
    return nc


def _prep_inputs(x: np.ndarray, I_i: np.ndarray):
    """Host-side layout: x transposed; per-core per-queue-window idx tensors."""
    tbl = np.ascontiguousarray(x.astype(np.float32, copy=False).T)  # [G, B]
    idx_maps = []
    I = np.asarray(I_i)
    for k in range(NCORES):
        Ik = I[k * GSH : (k + 1) * GSH]  # [256, 32, 8] values in [0, G)
        Ikr = Ik.reshape(128, 2, S, L)  # [p, gl, s, l]
        # grp c2 = gl*32 + s; call (lb, l) covers grps 16lb..16lb+16
        grp = np.transpose(Ikr, (1, 2, 3, 0)).reshape(2 * S, L, 128)  # [c2, l, p]
        idx_w = np.empty((16, IDX_COLS), dtype=np.int16)
        for c in range(NBATCH * L):
            lb, l = c // L, c % L
            flat = grp[
                lb * GRP_PER_BATCH : (lb + 1) * GRP_PER_BATCH, l, :
            ].reshape(NIDX)  # j = i2*128+p
            W = flat.reshape(COLS_PER_CALL, 16).T.astype(np.int16)  # [r, col]
            idx_w[:, c * COLS_PER_CALL : (c + 1) * COLS_PER_CALL] = W
        idx_maps.append(np.tile(idx_w, (8, 1)))
    return tbl, idx_maps


def kernel(x: np.ndarray, I_i: np.ndarray) -> np.ndarray:
    global _nc_cache, last_result
    if _nc_cache is None:
        _nc_cache = _build_nc()
    nc = _nc_cache
    tbl, idx_maps = _prep_inputs(x, I_i)
    in_maps = [{"tbl": tbl, "idx": idx_maps[k]} for k in range(NCORES)]
    res = run_bass_kernel_spmd(nc, in_maps, core_ids=list(range(NCORES)))
    last_result = res
    C = np.empty((B, G), dtype=np.float32)
    for k in range(NCORES):
        o = res.results[k]["c"].reshape(128, 2, B)  # [p, gl, b]
        C[:, k * GSH : (k + 1) * GSH] = np.transpose(o, (2, 0, 1)).reshape(B, GSH)
    return C
